# revision 6
# baseline (speedup 1.0000x reference)
"""Causal self-attention (B=2, T=2048, C=1024, H=16) on 8 trn2 NeuronCores.

Sharding: data-parallel over B (2) x tensor-parallel over head groups (4),
so each of the 8 cores handles one batch element and 4 heads end-to-end:
QKV projection (its W_attn column slice), full-T causal attention for its
4 heads, and the partial output projection (its W_proj row slice). The
host sums the 4 per-batch partials and adds biases.

Device dataflow (all matmuls bf16; host pre-transposes and pre-casts):
  x^T uploaded directly as bf16 [C, T] (no on-device transpose)
  QKV(tg): Q^T/K^T ([d, t] bf16) and V ([t, d] bf16) per 512-token group
  S^T[k, q] = K^T.T @ Q^T per head (causal block-skipped + trimmed)
  P = exp(S/8) on ScalarE (bf16), diagonal-block mask on VectorE
  y^T = (V|1).T @ P^T accumulated in PSUM (rowsum rides along)
  y^T *= 1/rowsum (reciprocal batched via partition-reshape DMAs)
  out^T = Wp_local.T @ y^T -> DRAM bf16 [1024, 2048] per core

Software pipeline (qi-outer): QKV(tg+1) and proj(tg-1) matmuls are
interleaved as filler into attention(qi=tg)'s S->exp->PV loop so the PE
never starves while ScalarE runs exp.
"""

import numpy as np
import ml_dtypes

import concourse.bass as bass
import concourse.mybir as mybir
import concourse.tile as tile
from concourse import bacc
from concourse.bass_utils import run_bass_kernel_spmd

F32 = mybir.dt.float32
F32R = mybir.dt.float32r
BF16 = mybir.dt.bfloat16
AF = mybir.ActivationFunctionType

B, T, C, H = 2, 2048, 1024, 16
HD = C // H          # 64
NCORES = 8
CTILES = C // 128    # 8 contraction chunks
TT = T // 128        # 16 token tiles of 128
QG = T // 512        # 4 q-groups of 512
SCL = 1.0 / float(np.sqrt(HD))


def build_nc():
    nc = bacc.Bacc("TRN2", target_bir_lowering=False)

    xT_d = nc.declare_dram_parameter("xT_b", [C, T], BF16, isOutput=False)
    w_d = nc.declare_dram_parameter("w_l", [C, 768], BF16, isOutput=False)
    bqk_d = nc.declare_dram_parameter("b_qk", [4, 128], F32, isOutput=False)
    wp_d = nc.declare_dram_parameter("wp_l", [256, C], BF16, isOutput=False)
    out_d = nc.declare_dram_parameter("out_T", [C, T], BF16, isOutput=True)

    with tile.TileContext(nc) as tc:
        with tc.tile_pool(name="persist", bufs=1) as pp:
            _build_body(nc, tc, pp, xT_d, w_d, bqk_d, wp_d, out_d)
    nc.compile()
    return nc


def _build_body(nc, tc, pp, xT_d, w_d, bqk_d, wp_d, out_d):
    # ---- constants ----
    # causal mask for S^T diagonal blocks: keep where q(col) >= k(row)
    m0 = pp.tile([128, 128], BF16, tag="m0")
    nc.gpsimd.memset(m0, 1.0)
    nc.gpsimd.affine_select(out=m0, in_=m0, compare_op=mybir.AluOpType.is_ge,
                            fill=0.0, base=0, pattern=[[1, 128]], channel_multiplier=-1)

    b_sb = pp.tile([128, 4], F32, tag="b_sb")

    # ---- persistent tiles ----
    wt_all = pp.tile([128, CTILES * 768], BF16, tag="wt_all")
    wt = [wt_all[:, 768 * ci:768 * (ci + 1)] for ci in range(CTILES)]
    wp_all = pp.tile([128, 2 * C], BF16, tag="wp_all")
    wp = [wp_all[:, C * k:C * (k + 1)] for k in range(2)]
    xT_all = pp.tile([128, CTILES * T], BF16, tag="xT_all")
    xT = [xT_all[:, T * ci:T * (ci + 1)] for ci in range(CTILES)]
    qk_pair = [pp.tile([128, T], BF16, tag=f"qkp{j}", name=f"qkp{j}") for j in range(4)]
    vt = [pp.tile([128, 260], BF16, tag=f"v{t}", name=f"v{t}") for t in range(TT)]
    y_un = [pp.tile([128, T], BF16, tag=f"y{hp}", name=f"y{hp}") for hp in range(2)]

    for t in range(TT):
        nc.vector.memset(vt[t].rearrange("p (h c) -> p h c", c=65)[:, :, 64:65], 1.0)

    def QT(h):
        lo = 64 * (h % 2)
        return qk_pair[h // 2][lo:lo + 64, :]

    def KT(h):
        lo = 64 * (h % 2)
        return qk_pair[2 + h // 2][lo:lo + 64, :]

    def load_xT(tg):
        nc.sync.dma_start(
            out=xT_all.rearrange("p (ci t) -> p ci t", ci=CTILES)[:, :, 512 * tg:512 * (tg + 1)],
            in_=xT_d.ap().rearrange("(ci p) t -> p ci t", ci=CTILES)[:, :, 512 * tg:512 * (tg + 1)])

    # startup DMAs: weights first, then first x slice, then small constants
    nc.sync.dma_start(out=wt_all.rearrange("p (ci j) -> p ci j", ci=CTILES),
                      in_=w_d.ap().rearrange("(ci p) j -> p ci j", ci=CTILES))
    load_xT(0)
    nc.sync.dma_start(out=wp_all.rearrange("p (k j) -> p k j", k=2),
                      in_=wp_d.ap().rearrange("(k p) j -> p k j", k=2))
    nc.sync.dma_start(out=b_sb, in_=bqk_d.ap().rearrange("j p -> p j"))

    with tc.tile_pool(name="qv", bufs=2, space="PSUM") as qv_pool, \
         tc.tile_pool(name="s", bufs=2, space="PSUM") as s_pool, \
         tc.tile_pool(name="y", bufs=1, space="PSUM") as y_pool, \
         tc.tile_pool(name="pb", bufs=5) as p_pool, \
         tc.tile_pool(name="stg", bufs=2) as st_pool, \
         tc.tile_pool(name="rr", bufs=2) as rr_pool, \
         tc.tile_pool(name="osb", bufs=8) as osb_pool:

        state = {}

        def qkv_steps(tg):
            """One step per contraction-chunk matmul; Q/K blocks then V tiles."""
            steps = []
            for jc in range(4):
                for ci in range(CTILES):
                    def step(jc=jc, ci=ci, tg=tg):
                        if ci == 0:
                            state[("qk", jc)] = qv_pool.tile([128, 512], F32, tag="qv", name=f"qk{jc}")
                        ps = state[("qk", jc)]
                        nc.tensor.matmul(ps, wt[ci][:, 128 * jc:128 * (jc + 1)],
                                         xT[ci][:, 512 * tg:512 * (tg + 1)],
                                         start=(ci == 0), stop=(ci == CTILES - 1))
                        if ci == CTILES - 1:
                            nc.scalar.activation(qk_pair[jc][:, 512 * tg:512 * (tg + 1)],
                                                 ps, AF.Identity,
                                                 bias=b_sb[:, jc:jc + 1], scale=1.0)
                    steps.append(step)
            for tq in range(4):
                t = 4 * tg + tq
                for ci in range(CTILES):
                    def step(t=t, ci=ci):
                        if ci == 0:
                            state[("v", t)] = qv_pool.tile([128, 512], F32, tag="qv", name=f"pv{t}")
                        pv = state[("v", t)]
                        nc.tensor.matmul(pv[:, 0:256], xT[ci][:, 128 * t:128 * (t + 1)],
                                         wt[ci][:, 512:768],
                                         start=(ci == 0), stop=(ci == CTILES - 1))
                        if ci == CTILES - 1:
                            nc.vector.tensor_copy(
                                vt[t].rearrange("p (h c) -> p h c", c=65)[:, :, 0:64],
                                pv[:, 0:256].rearrange("p (h c) -> p h c", c=64))
                    steps.append(step)
            return steps

        def proj_steps(tg):
            steps = []
            for cg in range(2):
                def alloc(cg=cg):
                    state[("osb", cg)] = osb_pool.tile([128, 2048], BF16, tag="osb",
                                                       name=f"osb{cg}")
                for cq in range(4):
                    co = 4 * cg + cq
                    def mm(co=co, tg=tg, cg=cg, cq=cq, alloc=alloc):
                        if cq == 0:
                            alloc()
                        pr = qv_pool.tile([128, 512], F32, tag="qv", name=f"pr{co}")
                        state[("pr", co)] = pr
                        nc.tensor.matmul(pr, wp[0][:, 128 * co:128 * (co + 1)],
                                         y_un[0][:, 512 * tg:512 * (tg + 1)],
                                         start=True, stop=False)
                        nc.tensor.matmul(pr, wp[1][:, 128 * co:128 * (co + 1)],
                                         y_un[1][:, 512 * tg:512 * (tg + 1)],
                                         start=False, stop=True)

                    def wr(co=co, tg=tg, cg=cg, cq=cq):
                        pr = state[("pr", co)]
                        osb = state[("osb", cg)]
                        dst = osb[:, 512 * cq:512 * (cq + 1)]
                        if co % 2 == 0:
                            nc.scalar.copy(dst, pr)
                        else:
                            nc.vector.tensor_copy(dst, pr)
                        if cq == 3:
                            nc.sync.dma_start(
                                out=out_d.ap().rearrange("(j p) t -> p j t", j=CTILES)[
                                    :, 4 * cg:4 * cg + 4, 512 * tg:512 * (tg + 1)],
                                in_=osb.rearrange("p (j t) -> p j t", j=4))
                    steps.append(mm)
                    steps.append(wr)
            return steps

        def emit_pv(psA, psB, ki, pAB, hA, hB, woff, st, sp):
            nc.tensor.matmul(psA[0:65, woff:512],
                             vt[ki][:, 65 * hA:65 * hA + 65],
                             pAB[:, woff:512], start=st, stop=sp)
            nc.tensor.matmul(psB[0:65, woff:512],
                             vt[ki][:, 65 * hB:65 * hB + 65],
                             pAB[:, 512 + woff:1024], start=st, stop=sp)

        def epilogue(qi, hp, psA, psB):
            # 1/rowsum in place on the psum ride-along row, broadcast across
            # partitions on Pool, then one fused psum*recip multiply per head.
            rr = rr_pool.tile([128, 1024], F32, tag="rr")
            with nc.allow_low_precision(reason="softmax denominator reciprocal"):
                nc.vector.reciprocal(rr[64:65, 0:512], psA[64:65, :])
                nc.vector.reciprocal(rr[64:65, 512:1024], psB[64:65, :])
            rr0 = rr_pool.tile([1, 1024], F32, tag="rr0")
            nc.sync.dma_start(out=rr0, in_=rr[64:65, :])
            rbA = rr_pool.tile([128, 512], F32, tag="rbA")
            rbB = rr_pool.tile([128, 512], F32, tag="rbB")
            nc.gpsimd.partition_broadcast(rbA, rr0[0:1, 0:512], channels=64)
            nc.gpsimd.partition_broadcast(rbB, rr0[0:1, 512:1024], channels=128)
            stB = st_pool.tile([128, 512], BF16, tag="st", name="stB")
            with nc.allow_low_precision(reason="softmax normalize in bf16"):
                nc.vector.tensor_mul(y_un[hp][0:64, 512 * qi:512 * (qi + 1)],
                                     psA[0:64, :], rbA[0:64, :])
                nc.vector.tensor_mul(stB[0:64, :], psB[0:64, :], rbB[64:128, :])
            nc.sync.dma_start(out=y_un[hp][64:128, 512 * qi:512 * (qi + 1)],
                              in_=stB[0:64, :])

        def attention(qi, filler):
            nkt = 4 * qi + 4
            slots = 2 * nkt
            nf = len(filler)
            prog = {"done": 0, "slot": 0}

            def pop_for_slot():
                prog["slot"] += 1
                target = (nf * prog["slot"]) // slots
                while prog["done"] < target:
                    filler[prog["done"]]()
                    prog["done"] += 1

            for hp in range(2):
                hA, hB = 2 * hp, 2 * hp + 1
                psA = y_pool.tile([128, 512], F32, tag="yA")
                psB = y_pool.tile([128, 512], F32, tag="yB")
                pend = None
                for ki in range(nkt):
                    r = ki - 4 * qi
                    soff = 0 if r < 1 else 128 * r
                    sAB = s_pool.tile([128, 1024], F32, tag="s")
                    for half, h in ((0, hA), (1, hB)):
                        nc.tensor.matmul(
                            sAB[:, 512 * half + soff:512 * half + 512],
                            KT(h)[:, 128 * ki:128 * (ki + 1)],
                            QT(h)[:, 512 * qi + soff:512 * (qi + 1)],
                            start=True, stop=True)
                    pAB = p_pool.tile([128, 1024], BF16, tag="p")
                    if r >= 1:
                        we = 128 * r
                        nc.scalar.activation(
                            pAB.rearrange("p (h q) -> p h q", h=2)[:, :, we:512],
                            sAB.rearrange("p (h q) -> p h q", h=2)[:, :, we:512],
                            AF.Exp, scale=SCL)
                    else:
                        nc.scalar.activation(pAB, sAB, AF.Exp, scale=SCL)
                    if r >= 0:
                        for half in range(2):
                            base = 512 * half + 128 * r
                            nc.vector.tensor_mul(pAB[:, base:base + 128],
                                                 pAB[:, base:base + 128], m0)
                    pop_for_slot()
                    if pend is not None:
                        emit_pv(*pend)
                    pend = (psA, psB, ki, pAB, hA, hB, 0 if r < 0 else 128 * r,
                            ki == 0, ki == nkt - 1)
                emit_pv(*pend)
                epilogue(qi, hp, psA, psB)

        # ---- pipelined main sequence ----
        for step in qkv_steps(0):
            step()
        load_xT(1)
        filler_map = {
            0: lambda: qkv_steps(1),
            1: lambda: qkv_steps(2),
            2: lambda: qkv_steps(3) + proj_steps(0),
            3: lambda: proj_steps(1) + proj_steps(2),
        }
        for qi in range(QG):
            if qi + 2 < QG:
                load_xT(qi + 2)
            attention(qi, filler_map[qi]())
        for step in proj_steps(QG - 1):
            step()


_NC = None


def _get_nc():
    global _NC
    if _NC is None:
        _NC = build_nc()
    return _NC


def kernel(x, W_attn, b_attn, W_proj, b_proj, _trace=False):
    x = np.asarray(x, dtype=np.float32)
    W_attn = np.asarray(W_attn, dtype=np.float32)
    b_attn = np.asarray(b_attn, dtype=np.float32)
    W_proj = np.asarray(W_proj, dtype=np.float32)
    b_proj = np.asarray(b_proj, dtype=np.float32)

    BF = ml_dtypes.bfloat16
    xTs = [np.ascontiguousarray(x[b].T.astype(BF)) for b in range(B)]
    in_maps = []
    for core in range(NCORES):
        b, hg = divmod(core, 4)
        qs = [W_attn[:, 64 * (4 * hg + h):64 * (4 * hg + h + 1)] for h in range(4)]
        ks = [W_attn[:, C + 64 * (4 * hg + h):C + 64 * (4 * hg + h + 1)] for h in range(4)]
        vs = [W_attn[:, 2 * C + 64 * (4 * hg + h):2 * C + 64 * (4 * hg + h + 1)] for h in range(4)]
        w_l = np.concatenate(qs + ks + vs, axis=1).astype(BF)
        bq = [b_attn[64 * (4 * hg + h):64 * (4 * hg + h + 1)] for h in range(4)]
        bk = [b_attn[C + 64 * (4 * hg + h):C + 64 * (4 * hg + h + 1)] for h in range(4)]
        b_qk = np.stack([np.concatenate(bq[0:2]), np.concatenate(bq[2:4]),
                         np.concatenate(bk[0:2]), np.concatenate(bk[2:4])])
        wp_l = np.concatenate(
            [W_proj[64 * (4 * hg + h):64 * (4 * hg + h + 1), :] for h in range(4)],
            axis=0).astype(BF)
        in_maps.append({
            "xT_b": xTs[b],
            "w_l": np.ascontiguousarray(w_l),
            "b_qk": np.ascontiguousarray(b_qk.astype(np.float32)),
            "wp_l": np.ascontiguousarray(wp_l),
        })

    nc = _get_nc()
    kwargs = {}
    if _trace:
        kwargs = dict(trace=True, trace_cores=[0])
    res = run_bass_kernel_spmd(nc, in_maps, core_ids=list(range(NCORES)), **kwargs)

    # V-bias folds into the output bias because softmax rows sum to 1.
    bias_total = b_proj + b_attn[2 * C:3 * C] @ W_proj
    out = np.empty((B, T, C), dtype=np.float32)
    for b in range(B):
        acc = res.results[4 * b]["out_T"].astype(np.float32)
        for hg in range(1, 4):
            acc = acc + res.results[4 * b + hg]["out_T"].astype(np.float32)
        out[b] = acc.T + bias_total[None, :]
    if _trace:
        return out, res
    return out


# revision 8
# speedup vs baseline: 1.0588x; 1.0588x over previous
"""Causal self-attention (B=2, T=2048, C=1024, H=16) on 8 trn2 NeuronCores.

Sharding: data-parallel over B (2) x tensor-parallel over head groups (4),
so each of the 8 cores handles one batch element and 4 heads end-to-end:
QKV projection (its W_attn column slice), full-T causal attention for its
4 heads, and the partial output projection (its W_proj row slice). The
host sums the 4 per-batch partials and adds biases.

Device dataflow (all matmuls bf16; host pre-transposes and pre-casts):
  x^T uploaded directly as bf16 [C, T] (no on-device transpose)
  QKV(tg): Q^T/K^T ([d, t] bf16) and V ([t, d] bf16) per 512-token group
  S^T[k, q] = K^T.T @ Q^T per head (causal block-skipped + trimmed)
  P = exp(S/8) on ScalarE (bf16), diagonal-block mask on VectorE
  y^T = (V|1).T @ P^T accumulated in PSUM (rowsum rides along)
  y^T *= 1/rowsum (reciprocal batched via partition-reshape DMAs)
  out^T = Wp_local.T @ y^T -> DRAM bf16 [1024, 2048] per core

Software pipeline (qi-outer): QKV(tg+1) and proj(tg-1) matmuls are
interleaved as filler into attention(qi=tg)'s S->exp->PV loop so the PE
never starves while ScalarE runs exp.
"""

import numpy as np
import ml_dtypes

import concourse.bass as bass
import concourse.mybir as mybir
import concourse.tile as tile
from concourse import bacc
from concourse.bass_utils import run_bass_kernel_spmd

F32 = mybir.dt.float32
F32R = mybir.dt.float32r
BF16 = mybir.dt.bfloat16
AF = mybir.ActivationFunctionType

B, T, C, H = 2, 2048, 1024, 16
HD = C // H          # 64
NCORES = 8
CTILES = C // 128    # 8 contraction chunks
TT = T // 128        # 16 token tiles of 128
QG = T // 512        # 4 q-groups of 512
SCL = 1.0 / float(np.sqrt(HD))


def build_nc():
    nc = bacc.Bacc("TRN2", target_bir_lowering=False)

    xT_d = nc.declare_dram_parameter("xT_b", [C, T], BF16, isOutput=False)
    w_d = nc.declare_dram_parameter("w_l", [C, 768], BF16, isOutput=False)
    bqk_d = nc.declare_dram_parameter("b_qk", [4, 128], F32, isOutput=False)
    wp_d = nc.declare_dram_parameter("wp_l", [256, C], BF16, isOutput=False)
    out_d = nc.declare_dram_parameter("out_T", [C, T], BF16, isOutput=True)

    with tile.TileContext(nc) as tc:
        with tc.tile_pool(name="persist", bufs=1) as pp:
            _build_body(nc, tc, pp, xT_d, w_d, bqk_d, wp_d, out_d)
    nc.compile()
    return nc


def _build_body(nc, tc, pp, xT_d, w_d, bqk_d, wp_d, out_d):
    # ---- constants ----
    # causal mask for S^T diagonal blocks: keep where q(col) >= k(row)
    m0 = pp.tile([128, 128], BF16, tag="m0")
    nc.gpsimd.memset(m0, 1.0)
    nc.gpsimd.affine_select(out=m0, in_=m0, compare_op=mybir.AluOpType.is_ge,
                            fill=0.0, base=0, pattern=[[1, 128]], channel_multiplier=-1)

    b_sb = pp.tile([128, 4], F32, tag="b_sb")

    # ---- persistent tiles ----
    wt_all = pp.tile([128, CTILES * 768], BF16, tag="wt_all")
    wt = [wt_all[:, 768 * ci:768 * (ci + 1)] for ci in range(CTILES)]
    wp_all = pp.tile([128, 2 * C], BF16, tag="wp_all")
    wp = [wp_all[:, C * k:C * (k + 1)] for k in range(2)]
    xT_all = pp.tile([128, CTILES * T], BF16, tag="xT_all")
    xT = [xT_all[:, T * ci:T * (ci + 1)] for ci in range(CTILES)]
    qk_pair = [pp.tile([128, T], BF16, tag=f"qkp{j}", name=f"qkp{j}") for j in range(4)]
    vt = [pp.tile([128, 260], BF16, tag=f"v{t}", name=f"v{t}") for t in range(TT)]
    y_un = [pp.tile([128, T], BF16, tag=f"y{hp}", name=f"y{hp}") for hp in range(2)]

    for t in range(TT):
        g = vt[t].rearrange("p (g c) -> p g c", c=130)
        nc.vector.memset(g[:, :, 64:65], 1.0)   # even heads: ones col last
        nc.vector.memset(g[:, :, 65:66], 1.0)   # odd heads: ones col first

    def QT(h):
        lo = 64 * (h % 2)
        return qk_pair[h // 2][lo:lo + 64, :]

    def KT(h):
        lo = 64 * (h % 2)
        return qk_pair[2 + h // 2][lo:lo + 64, :]

    def VT(t, h):
        g = h // 2
        if h % 2 == 0:
            return vt[t][:, 130 * g:130 * g + 65]       # [dims64 | ones]
        return vt[t][:, 130 * g + 65:130 * g + 130]     # [ones | dims64]

    def load_xT(tg):
        nc.sync.dma_start(
            out=xT_all.rearrange("p (ci t) -> p ci t", ci=CTILES)[:, :, 512 * tg:512 * (tg + 1)],
            in_=xT_d.ap().rearrange("(ci p) t -> p ci t", ci=CTILES)[:, :, 512 * tg:512 * (tg + 1)])

    # startup DMAs: small constants, then first weight/x chunks so QKV(0)
    # starts early, then the rest
    nc.sync.dma_start(out=b_sb, in_=bqk_d.ap().rearrange("j p -> p j"))
    wre = wt_all.rearrange("p (ci j) -> p ci j", ci=CTILES)
    wsrc = w_d.ap().rearrange("(ci p) j -> p ci j", ci=CTILES)
    xre = xT_all.rearrange("p (ci t) -> p ci t", ci=CTILES)
    xsrc = xT_d.ap().rearrange("(ci p) t -> p ci t", ci=CTILES)
    nc.sync.dma_start(out=wre[:, 0:2, :], in_=wsrc[:, 0:2, :])
    nc.sync.dma_start(out=xre[:, 0:2, 0:512], in_=xsrc[:, 0:2, 0:512])
    nc.sync.dma_start(out=wre[:, 2:CTILES, :], in_=wsrc[:, 2:CTILES, :])
    nc.sync.dma_start(out=xre[:, 2:CTILES, 0:512], in_=xsrc[:, 2:CTILES, 0:512])
    nc.sync.dma_start(out=wp_all.rearrange("p (k j) -> p k j", k=2),
                      in_=wp_d.ap().rearrange("(k p) j -> p k j", k=2))

    with tc.tile_pool(name="qv", bufs=2, space="PSUM") as qv_pool, \
         tc.tile_pool(name="s", bufs=2, space="PSUM") as s_pool, \
         tc.tile_pool(name="y", bufs=1, space="PSUM") as y_pool, \
         tc.tile_pool(name="pb", bufs=5) as p_pool, \
         tc.tile_pool(name="stg", bufs=2) as st_pool, \
         tc.tile_pool(name="rr", bufs=2) as rr_pool, \
         tc.tile_pool(name="osb", bufs=8) as osb_pool:

        state = {}

        def qkv_steps(tg, jc_order=(0, 1, 2, 3)):
            """One step per contraction-chunk matmul; Q/K blocks then V tiles."""
            steps = []
            for jc in jc_order:
                for ci in range(CTILES):
                    def step(jc=jc, ci=ci, tg=tg):
                        if ci == 0:
                            state[("qk", jc)] = qv_pool.tile([128, 512], F32, tag="qv", name=f"qk{jc}")
                        ps = state[("qk", jc)]
                        nc.tensor.matmul(ps, wt[ci][:, 128 * jc:128 * (jc + 1)],
                                         xT[ci][:, 512 * tg:512 * (tg + 1)],
                                         start=(ci == 0), stop=(ci == CTILES - 1))
                        if ci == CTILES - 1:
                            nc.scalar.activation(qk_pair[jc][:, 512 * tg:512 * (tg + 1)],
                                                 ps, AF.Identity,
                                                 bias=b_sb[:, jc:jc + 1], scale=1.0)
                    steps.append(step)
            for tq in range(4):
                t = 4 * tg + tq
                for ci in range(CTILES):
                    def step(t=t, ci=ci):
                        if ci == 0:
                            state[("v", t)] = qv_pool.tile([128, 512], F32, tag="qv", name=f"pv{t}")
                        pv = state[("v", t)]
                        nc.tensor.matmul(pv[:, 0:256], xT[ci][:, 128 * t:128 * (t + 1)],
                                         wt[ci][:, 512:768],
                                         start=(ci == 0), stop=(ci == CTILES - 1))
                        if ci == CTILES - 1:
                            g = vt[t].rearrange("p (g c) -> p g c", c=130)
                            pg = pv[:, 0:256].rearrange("p (g c) -> p g c", c=128)
                            nc.vector.tensor_copy(g[:, :, 0:64], pg[:, :, 0:64])
                            nc.vector.tensor_copy(g[:, :, 66:130], pg[:, :, 64:128])
                    steps.append(step)
            return steps

        def proj_steps(tg):
            steps = []
            for cg in range(2):
                def alloc(cg=cg):
                    state[("osb", cg)] = osb_pool.tile([128, 2048], BF16, tag="osb",
                                                       name=f"osb{cg}")
                for cq in range(4):
                    co = 4 * cg + cq
                    def mm(co=co, tg=tg, cg=cg, cq=cq, alloc=alloc):
                        if cq == 0:
                            alloc()
                        pr = qv_pool.tile([128, 512], F32, tag="qv", name=f"pr{co}")
                        state[("pr", co)] = pr
                        nc.tensor.matmul(pr, wp[0][:, 128 * co:128 * (co + 1)],
                                         y_un[0][:, 512 * tg:512 * (tg + 1)],
                                         start=True, stop=False)
                        nc.tensor.matmul(pr, wp[1][:, 128 * co:128 * (co + 1)],
                                         y_un[1][:, 512 * tg:512 * (tg + 1)],
                                         start=False, stop=True)

                    def wr(co=co, tg=tg, cg=cg, cq=cq):
                        pr = state[("pr", co)]
                        osb = state[("osb", cg)]
                        dst = osb[:, 512 * cq:512 * (cq + 1)]
                        nc.vector.tensor_copy(dst, pr)
                        if cq == 3:
                            nc.sync.dma_start(
                                out=out_d.ap().rearrange("(j p) t -> p j t", j=CTILES)[
                                    :, 4 * cg:4 * cg + 4, 512 * tg:512 * (tg + 1)],
                                in_=osb.rearrange("p (j t) -> p j t", j=4))
                    steps.append(mm)
                    steps.append(wr)
            return steps

        def emit_pv(psA, psB, ki, pAB, hA, hB, woff, st, sp):
            nc.tensor.matmul(psA[0:65, woff:512], VT(ki, hA),
                             pAB[:, woff:512], start=st, stop=sp)
            nc.tensor.matmul(psB[0:65, woff:512], VT(ki, hB),
                             pAB[:, 512 + woff:1024], start=st, stop=sp)

        def epilogue(qi, hp, psA, psB):
            # head A: rowsum rides at psum row 64 -> recip -> bounce to row 0
            # via tiny DMA -> Pool broadcast. head B: ones-first layout puts
            # its rowsum at psum row 0 -> recip straight into the broadcast
            # source, no DMA. One fused psum*recip multiply per head.
            rr = rr_pool.tile([128, 512], F32, tag="rr")
            rr0 = rr_pool.tile([1, 1024], F32, tag="rr0")
            with nc.allow_low_precision(reason="softmax denominator reciprocal"):
                nc.vector.reciprocal(rr[64:65, :], psA[64:65, :])
                nc.vector.reciprocal(rr0[0:1, 512:1024], psB[0:1, :])
            nc.sync.dma_start(out=rr0[0:1, 0:512], in_=rr[64:65, :])
            rbA = rr_pool.tile([128, 512], F32, tag="rbA")
            rbB = rr_pool.tile([128, 512], F32, tag="rbB")
            nc.gpsimd.partition_broadcast(rbA[0:64, :], rr0[0:1, 0:512], channels=64)
            nc.gpsimd.partition_broadcast(rbB[0:65, :], rr0[0:1, 512:1024], channels=65)
            stB = st_pool.tile([128, 512], BF16, tag="st", name="stB")
            with nc.allow_low_precision(reason="softmax normalize in bf16"):
                nc.vector.tensor_mul(y_un[hp][0:64, 512 * qi:512 * (qi + 1)],
                                     psA[0:64, :], rbA[0:64, :])
                nc.vector.tensor_mul(stB[0:65, :], psB[0:65, :], rbB[0:65, :])
            nc.sync.dma_start(out=y_un[hp][64:128, 512 * qi:512 * (qi + 1)],
                              in_=stB[1:65, :])

        def attention(qi, filler):
            nkt = 4 * qi + 4
            slots = 2 * nkt
            nf = len(filler)
            prog = {"done": 0, "slot": 0}

            def pop_for_slot():
                prog["slot"] += 1
                target = (nf * prog["slot"]) // slots
                while prog["done"] < target:
                    filler[prog["done"]]()
                    prog["done"] += 1

            for hp in range(2):
                hA, hB = 2 * hp, 2 * hp + 1
                psA = y_pool.tile([128, 512], F32, tag="yA")
                psB = y_pool.tile([128, 512], F32, tag="yB")
                pend = None
                for ki in range(nkt):
                    r = ki - 4 * qi
                    soff = 0 if r < 1 else 128 * r
                    sAB = s_pool.tile([128, 1024], F32, tag="s")
                    for half, h in ((0, hA), (1, hB)):
                        nc.tensor.matmul(
                            sAB[:, 512 * half + soff:512 * half + 512],
                            KT(h)[:, 128 * ki:128 * (ki + 1)],
                            QT(h)[:, 512 * qi + soff:512 * (qi + 1)],
                            start=True, stop=True)
                    pAB = p_pool.tile([128, 1024], BF16, tag="p")
                    if r >= 1:
                        we = 128 * r
                        nc.scalar.activation(
                            pAB.rearrange("p (h q) -> p h q", h=2)[:, :, we:512],
                            sAB.rearrange("p (h q) -> p h q", h=2)[:, :, we:512],
                            AF.Exp, scale=SCL)
                    else:
                        nc.scalar.activation(pAB, sAB, AF.Exp, scale=SCL)
                    if r >= 0:
                        for half in range(2):
                            base = 512 * half + 128 * r
                            nc.vector.tensor_mul(pAB[:, base:base + 128],
                                                 pAB[:, base:base + 128], m0)
                    pop_for_slot()
                    if pend is not None:
                        emit_pv(*pend)
                    pend = (psA, psB, ki, pAB, hA, hB, 0 if r < 0 else 128 * r,
                            ki == 0, ki == nkt - 1)
                emit_pv(*pend)
                epilogue(qi, hp, psA, psB)

        # ---- pipelined main sequence ----
        for step in qkv_steps(0, jc_order=(0, 2, 1, 3)):
            step()
        load_xT(1)
        filler_map = {
            0: lambda: qkv_steps(1),
            1: lambda: qkv_steps(2),
            2: lambda: qkv_steps(3) + proj_steps(0),
            3: lambda: proj_steps(1) + proj_steps(2),
        }
        for qi in range(QG):
            if qi + 2 < QG:
                load_xT(qi + 2)
            attention(qi, filler_map[qi]())
        for step in proj_steps(QG - 1):
            step()


_NC = None


def _get_nc():
    global _NC
    if _NC is None:
        _NC = build_nc()
    return _NC


def kernel(x, W_attn, b_attn, W_proj, b_proj, _trace=False):
    x = np.asarray(x, dtype=np.float32)
    W_attn = np.asarray(W_attn, dtype=np.float32)
    b_attn = np.asarray(b_attn, dtype=np.float32)
    W_proj = np.asarray(W_proj, dtype=np.float32)
    b_proj = np.asarray(b_proj, dtype=np.float32)

    BF = ml_dtypes.bfloat16
    xTs = [np.ascontiguousarray(x[b].T.astype(BF)) for b in range(B)]
    in_maps = []
    for core in range(NCORES):
        b, hg = divmod(core, 4)
        qs = [W_attn[:, 64 * (4 * hg + h):64 * (4 * hg + h + 1)] for h in range(4)]
        ks = [W_attn[:, C + 64 * (4 * hg + h):C + 64 * (4 * hg + h + 1)] for h in range(4)]
        vs = [W_attn[:, 2 * C + 64 * (4 * hg + h):2 * C + 64 * (4 * hg + h + 1)] for h in range(4)]
        w_l = np.concatenate(qs + ks + vs, axis=1).astype(BF)
        bq = [b_attn[64 * (4 * hg + h):64 * (4 * hg + h + 1)] for h in range(4)]
        bk = [b_attn[C + 64 * (4 * hg + h):C + 64 * (4 * hg + h + 1)] for h in range(4)]
        b_qk = np.stack([np.concatenate(bq[0:2]), np.concatenate(bq[2:4]),
                         np.concatenate(bk[0:2]), np.concatenate(bk[2:4])])
        wp_l = np.concatenate(
            [W_proj[64 * (4 * hg + h):64 * (4 * hg + h + 1), :] for h in range(4)],
            axis=0).astype(BF)
        in_maps.append({
            "xT_b": xTs[b],
            "w_l": np.ascontiguousarray(w_l),
            "b_qk": np.ascontiguousarray(b_qk.astype(np.float32)),
            "wp_l": np.ascontiguousarray(wp_l),
        })

    nc = _get_nc()
    kwargs = {}
    if _trace:
        kwargs = dict(trace=True, trace_cores=[0])
    res = run_bass_kernel_spmd(nc, in_maps, core_ids=list(range(NCORES)), **kwargs)

    # V-bias folds into the output bias because softmax rows sum to 1.
    bias_total = b_proj + b_attn[2 * C:3 * C] @ W_proj
    out = np.empty((B, T, C), dtype=np.float32)
    for b in range(B):
        acc = res.results[4 * b]["out_T"].astype(np.float32)
        for hg in range(1, 4):
            acc = acc + res.results[4 * b + hg]["out_T"].astype(np.float32)
        out[b] = acc.T + bias_total[None, :]
    if _trace:
        return out, res
    return out


# revision 9
# speedup vs baseline: 1.0928x; 1.0322x over previous
"""Causal self-attention (B=2, T=2048, C=1024, H=16) on 8 trn2 NeuronCores.

Sharding: data-parallel over B (2) x tensor-parallel over head groups (4),
so each of the 8 cores handles one batch element and 4 heads end-to-end:
QKV projection (its W_attn column slice), full-T causal attention for its
4 heads, and the partial output projection (its W_proj row slice). The
host sums the 4 per-batch partials and adds biases.

Device dataflow (all matmuls bf16; host pre-transposes and pre-casts):
  x^T uploaded directly as bf16 [C, T] (no on-device transpose)
  QKV(tg): Q^T/K^T ([d, t] bf16) and V ([t, d] bf16) per 512-token group
  S^T[k, q] = K^T.T @ Q^T per head (causal block-skipped + trimmed)
  P = exp(S/8) on ScalarE (bf16), diagonal-block mask on VectorE
  y^T = (V|1).T @ P^T accumulated in PSUM (rowsum rides along)
  y^T *= 1/rowsum (reciprocal batched via partition-reshape DMAs)
  out^T = Wp_local.T @ y^T -> DRAM bf16 [1024, 2048] per core

Software pipeline (qi-outer): QKV(tg+1) and proj(tg-1) matmuls are
interleaved as filler into attention(qi=tg)'s S->exp->PV loop so the PE
never starves while ScalarE runs exp.
"""

import numpy as np
import ml_dtypes

import concourse.bass as bass
import concourse.mybir as mybir
import concourse.tile as tile
from concourse import bacc
from concourse.bass_utils import run_bass_kernel_spmd

F32 = mybir.dt.float32
F32R = mybir.dt.float32r
BF16 = mybir.dt.bfloat16
AF = mybir.ActivationFunctionType

B, T, C, H = 2, 2048, 1024, 16
HD = C // H          # 64
NCORES = 8
CTILES = C // 128    # 8 contraction chunks
TT = T // 128        # 16 token tiles of 128
QG = T // 512        # 4 q-groups of 512
SCL = 1.0 / float(np.sqrt(HD))


def build_nc():
    nc = bacc.Bacc("TRN2", target_bir_lowering=False)

    xT_d = nc.declare_dram_parameter("xT_b", [C, T], BF16, isOutput=False)
    w_d = nc.declare_dram_parameter("w_l", [C, 768], BF16, isOutput=False)
    bqk_d = nc.declare_dram_parameter("b_qk", [4, 128], F32, isOutput=False)
    wp_d = nc.declare_dram_parameter("wp_l", [256, C], BF16, isOutput=False)
    out_d = nc.declare_dram_parameter("out_T", [C, T], BF16, isOutput=True)

    with tile.TileContext(nc) as tc:
        with tc.tile_pool(name="persist", bufs=1) as pp:
            _build_body(nc, tc, pp, xT_d, w_d, bqk_d, wp_d, out_d)
    nc.compile()
    return nc


def _build_body(nc, tc, pp, xT_d, w_d, bqk_d, wp_d, out_d):
    # ---- constants ----
    # causal mask for S^T diagonal blocks: keep where q(col) >= k(row)
    m0 = pp.tile([128, 128], BF16, tag="m0")
    nc.gpsimd.memset(m0, 1.0)
    nc.gpsimd.affine_select(out=m0, in_=m0, compare_op=mybir.AluOpType.is_ge,
                            fill=0.0, base=0, pattern=[[1, 128]], channel_multiplier=-1)

    b_sb = pp.tile([128, 4], F32, tag="b_sb")
    scr = pp.tile([1, 4], F32, tag="scr")
    nc.scalar.activation(scr, scr, AF.Exp, scale=1.0)  # preload act table early

    # ---- persistent tiles ----
    wt_all = pp.tile([128, CTILES * 768], BF16, tag="wt_all")
    wt = [wt_all[:, 768 * ci:768 * (ci + 1)] for ci in range(CTILES)]
    wp_all = pp.tile([128, 2 * C], BF16, tag="wp_all")
    wp = [wp_all[:, C * k:C * (k + 1)] for k in range(2)]
    xT_all = pp.tile([128, CTILES * T], BF16, tag="xT_all")
    xT = [xT_all[:, T * ci:T * (ci + 1)] for ci in range(CTILES)]
    qk_pair = [pp.tile([128, T], BF16, tag=f"qkp{j}", name=f"qkp{j}") for j in range(4)]
    vt = [pp.tile([128, 260], BF16, tag=f"v{t}", name=f"v{t}") for t in range(TT)]
    y_un = [pp.tile([128, T], BF16, tag=f"y{hp}", name=f"y{hp}") for hp in range(2)]

    for t in range(TT):
        g = vt[t].rearrange("p (g c) -> p g c", c=130)
        nc.vector.memset(g[:, :, 64:65], 1.0)   # even heads: ones col last
        nc.vector.memset(g[:, :, 65:66], 1.0)   # odd heads: ones col first

    def QT(h):
        lo = 64 * (h % 2)
        return qk_pair[h // 2][lo:lo + 64, :]

    def KT(h):
        lo = 64 * (h % 2)
        return qk_pair[2 + h // 2][lo:lo + 64, :]

    def VT(t, h):
        g = h // 2
        if h % 2 == 0:
            return vt[t][:, 130 * g:130 * g + 65]       # [dims64 | ones]
        return vt[t][:, 130 * g + 65:130 * g + 130]     # [ones | dims64]

    def load_xT(tg):
        nc.sync.dma_start(
            out=xT_all.rearrange("p (ci t) -> p ci t", ci=CTILES)[:, :, 512 * tg:512 * (tg + 1)],
            in_=xT_d.ap().rearrange("(ci p) t -> p ci t", ci=CTILES)[:, :, 512 * tg:512 * (tg + 1)])

    # startup DMAs: small constants, then first weight/x chunks so QKV(0)
    # starts early, then the rest
    wre = wt_all.rearrange("p (ci j) -> p ci j", ci=CTILES)
    wsrc = w_d.ap().rearrange("(ci p) j -> p ci j", ci=CTILES)
    xre = xT_all.rearrange("p (ci t) -> p ci t", ci=CTILES)
    xsrc = xT_d.ap().rearrange("(ci p) t -> p ci t", ci=CTILES)
    nc.sync.dma_start(out=wre[:, 0:2, :], in_=wsrc[:, 0:2, :])
    nc.sync.dma_start(out=xre[:, 0:2, 0:512], in_=xsrc[:, 0:2, 0:512])
    nc.sync.dma_start(out=b_sb, in_=bqk_d.ap().rearrange("j p -> p j"))
    nc.sync.dma_start(out=wre[:, 2:CTILES, :], in_=wsrc[:, 2:CTILES, :])
    nc.sync.dma_start(out=xre[:, 2:CTILES, 0:512], in_=xsrc[:, 2:CTILES, 0:512])
    nc.sync.dma_start(out=wp_all.rearrange("p (k j) -> p k j", k=2),
                      in_=wp_d.ap().rearrange("(k p) j -> p k j", k=2))

    with tc.tile_pool(name="qv", bufs=2, space="PSUM") as qv_pool, \
         tc.tile_pool(name="s", bufs=2, space="PSUM") as s_pool, \
         tc.tile_pool(name="y", bufs=1, space="PSUM") as y_pool, \
         tc.tile_pool(name="pb", bufs=5) as p_pool, \
         tc.tile_pool(name="stg", bufs=2) as st_pool, \
         tc.tile_pool(name="rr", bufs=2) as rr_pool, \
         tc.tile_pool(name="osb", bufs=8) as osb_pool:

        state = {}

        def qkv_steps(tg, jc_order=(0, 1, 2, 3)):
            """One step per contraction-chunk matmul; Q/K blocks then V tiles."""
            steps = []
            for jc in jc_order:
                for ci in range(CTILES):
                    def step(jc=jc, ci=ci, tg=tg):
                        if ci == 0:
                            state[("qk", jc)] = qv_pool.tile([128, 512], F32, tag="qv", name=f"qk{jc}")
                        ps = state[("qk", jc)]
                        nc.tensor.matmul(ps, wt[ci][:, 128 * jc:128 * (jc + 1)],
                                         xT[ci][:, 512 * tg:512 * (tg + 1)],
                                         start=(ci == 0), stop=(ci == CTILES - 1))
                        if ci == CTILES - 1:
                            nc.scalar.activation(qk_pair[jc][:, 512 * tg:512 * (tg + 1)],
                                                 ps, AF.Identity,
                                                 bias=b_sb[:, jc:jc + 1], scale=1.0)
                    steps.append(step)
            for tq in range(4):
                t = 4 * tg + tq
                for ci in range(CTILES):
                    def step(t=t, ci=ci):
                        if ci == 0:
                            state[("v", t)] = qv_pool.tile([128, 512], F32, tag="qv", name=f"pv{t}")
                        pv = state[("v", t)]
                        nc.tensor.matmul(pv[:, 0:256], xT[ci][:, 128 * t:128 * (t + 1)],
                                         wt[ci][:, 512:768],
                                         start=(ci == 0), stop=(ci == CTILES - 1))
                        if ci == CTILES - 1:
                            g = vt[t].rearrange("p (g c) -> p g c", c=130)
                            pg = pv[:, 0:256].rearrange("p (g c) -> p g c", c=128)
                            nc.vector.tensor_copy(g[:, :, 0:64], pg[:, :, 0:64])
                            nc.vector.tensor_copy(g[:, :, 66:130], pg[:, :, 64:128])
                    steps.append(step)
            return steps

        def proj_steps(tg, fine=False):
            steps = []
            for cg in range(2):
                def alloc(cg=cg):
                    state[("osb", cg)] = osb_pool.tile([128, 2048], BF16, tag="osb",
                                                       name=f"osb{cg}")
                for cq in range(4):
                    co = 4 * cg + cq
                    def mm(co=co, tg=tg, cg=cg, cq=cq, alloc=alloc):
                        if cq == 0:
                            alloc()
                        pr = qv_pool.tile([128, 512], F32, tag="qv", name=f"pr{co}")
                        state[("pr", co)] = pr
                        nc.tensor.matmul(pr, wp[0][:, 128 * co:128 * (co + 1)],
                                         y_un[0][:, 512 * tg:512 * (tg + 1)],
                                         start=True, stop=False)
                        nc.tensor.matmul(pr, wp[1][:, 128 * co:128 * (co + 1)],
                                         y_un[1][:, 512 * tg:512 * (tg + 1)],
                                         start=False, stop=True)

                    def wr(co=co, tg=tg, cg=cg, cq=cq, fine=fine):
                        pr = state[("pr", co)]
                        osb = state[("osb", cg)]
                        dst = osb[:, 512 * cq:512 * (cq + 1)]
                        nc.vector.tensor_copy(dst, pr)
                        if fine:
                            nc.sync.dma_start(
                                out=out_d[128 * co:128 * (co + 1),
                                          512 * tg:512 * (tg + 1)],
                                in_=dst)
                        elif cq == 3:
                            nc.sync.dma_start(
                                out=out_d.ap().rearrange("(j p) t -> p j t", j=CTILES)[
                                    :, 4 * cg:4 * cg + 4, 512 * tg:512 * (tg + 1)],
                                in_=osb.rearrange("p (j t) -> p j t", j=4))
                    steps.append(mm)
                    steps.append(wr)
            return steps

        def emit_pv(psA, psB, ki, pAB, hA, hB, woff, st, sp):
            nc.tensor.matmul(psA[0:65, woff:512], VT(ki, hA),
                             pAB[:, woff:512], start=st, stop=sp)
            nc.tensor.matmul(psB[0:65, woff:512], VT(ki, hB),
                             pAB[:, 512 + woff:1024], start=st, stop=sp)

        def epilogue(qi, hp, psA, psB):
            # head A: rowsum rides at psum row 64 -> recip -> bounce to row 0
            # via tiny DMA -> Pool broadcast. head B: ones-first layout puts
            # its rowsum at psum row 0 -> recip straight into the broadcast
            # source, no DMA. One fused psum*recip multiply per head.
            rr = rr_pool.tile([128, 512], F32, tag="rr")
            rr0 = rr_pool.tile([1, 1024], F32, tag="rr0")
            with nc.allow_low_precision(reason="softmax denominator reciprocal"):
                nc.vector.reciprocal(rr[64:65, :], psA[64:65, :])
                nc.vector.reciprocal(rr0[0:1, 512:1024], psB[0:1, :])
            nc.sync.dma_start(out=rr0[0:1, 0:512], in_=rr[64:65, :])
            rbA = rr_pool.tile([128, 512], F32, tag="rbA")
            rbB = rr_pool.tile([128, 512], F32, tag="rbB")
            nc.gpsimd.partition_broadcast(rbA[0:64, :], rr0[0:1, 0:512], channels=64)
            nc.gpsimd.partition_broadcast(rbB[0:65, :], rr0[0:1, 512:1024], channels=65)
            stB = st_pool.tile([128, 512], BF16, tag="st", name="stB")
            with nc.allow_low_precision(reason="softmax normalize in bf16"):
                nc.vector.tensor_mul(y_un[hp][0:64, 512 * qi:512 * (qi + 1)],
                                     psA[0:64, :], rbA[0:64, :])
                nc.vector.tensor_mul(stB[0:65, :], psB[0:65, :], rbB[0:65, :])
            nc.sync.dma_start(out=y_un[hp][64:128, 512 * qi:512 * (qi + 1)],
                              in_=stB[1:65, :])

        def attention(qi, filler):
            nkt = 4 * qi + 4
            slots = 2 * nkt
            nf = len(filler)
            prog = {"done": 0, "slot": 0}

            def pop_for_slot():
                prog["slot"] += 1
                target = (nf * prog["slot"]) // slots
                while prog["done"] < target:
                    filler[prog["done"]]()
                    prog["done"] += 1

            for hp in range(2):
                hA, hB = 2 * hp, 2 * hp + 1
                psA = y_pool.tile([128, 512], F32, tag="yA")
                psB = y_pool.tile([128, 512], F32, tag="yB")
                pend = []
                for ki in range(nkt):
                    r = ki - 4 * qi
                    soff = 0 if r < 1 else 128 * r
                    sAB = s_pool.tile([128, 1024], F32, tag="s")
                    for half, h in ((0, hA), (1, hB)):
                        nc.tensor.matmul(
                            sAB[:, 512 * half + soff:512 * half + 512],
                            KT(h)[:, 128 * ki:128 * (ki + 1)],
                            QT(h)[:, 512 * qi + soff:512 * (qi + 1)],
                            start=True, stop=True)
                    pAB = p_pool.tile([128, 1024], BF16, tag="p")
                    if r >= 1:
                        we = 128 * r
                        nc.scalar.activation(
                            pAB.rearrange("p (h q) -> p h q", h=2)[:, :, we:512],
                            sAB.rearrange("p (h q) -> p h q", h=2)[:, :, we:512],
                            AF.Exp, scale=SCL)
                    else:
                        nc.scalar.activation(pAB, sAB, AF.Exp, scale=SCL)
                    if r >= 0:
                        for half in range(2):
                            base = 512 * half + 128 * r
                            nc.vector.tensor_mul(pAB[:, base:base + 128],
                                                 pAB[:, base:base + 128], m0)
                    pop_for_slot()
                    if len(pend) >= 2:
                        emit_pv(*pend.pop(0))
                    pend.append((psA, psB, ki, pAB, hA, hB, 0 if r < 0 else 128 * r,
                                 ki == 0, ki == nkt - 1))
                for p in pend:
                    emit_pv(*p)
                epilogue(qi, hp, psA, psB)

        # ---- pipelined main sequence ----
        for step in qkv_steps(0, jc_order=(0, 2, 1, 3)):
            step()
        load_xT(1)
        filler_map = {
            0: lambda: qkv_steps(1),
            1: lambda: qkv_steps(2),
            2: lambda: qkv_steps(3) + proj_steps(0),
            3: lambda: proj_steps(1) + proj_steps(2),
        }
        for qi in range(QG):
            if qi + 2 < QG:
                load_xT(qi + 2)
            attention(qi, filler_map[qi]())
        for step in proj_steps(QG - 1, fine=True):
            step()


_NC = None


def _get_nc():
    global _NC
    if _NC is None:
        _NC = build_nc()
    return _NC


def kernel(x, W_attn, b_attn, W_proj, b_proj, _trace=False):
    x = np.asarray(x, dtype=np.float32)
    W_attn = np.asarray(W_attn, dtype=np.float32)
    b_attn = np.asarray(b_attn, dtype=np.float32)
    W_proj = np.asarray(W_proj, dtype=np.float32)
    b_proj = np.asarray(b_proj, dtype=np.float32)

    BF = ml_dtypes.bfloat16
    xTs = [np.ascontiguousarray(x[b].T.astype(BF)) for b in range(B)]
    in_maps = []
    for core in range(NCORES):
        b, hg = divmod(core, 4)
        qs = [W_attn[:, 64 * (4 * hg + h):64 * (4 * hg + h + 1)] for h in range(4)]
        ks = [W_attn[:, C + 64 * (4 * hg + h):C + 64 * (4 * hg + h + 1)] for h in range(4)]
        vs = [W_attn[:, 2 * C + 64 * (4 * hg + h):2 * C + 64 * (4 * hg + h + 1)] for h in range(4)]
        w_l = np.concatenate(qs + ks + vs, axis=1).astype(BF)
        bq = [b_attn[64 * (4 * hg + h):64 * (4 * hg + h + 1)] for h in range(4)]
        bk = [b_attn[C + 64 * (4 * hg + h):C + 64 * (4 * hg + h + 1)] for h in range(4)]
        b_qk = np.stack([np.concatenate(bq[0:2]), np.concatenate(bq[2:4]),
                         np.concatenate(bk[0:2]), np.concatenate(bk[2:4])])
        wp_l = np.concatenate(
            [W_proj[64 * (4 * hg + h):64 * (4 * hg + h + 1), :] for h in range(4)],
            axis=0).astype(BF)
        in_maps.append({
            "xT_b": xTs[b],
            "w_l": np.ascontiguousarray(w_l),
            "b_qk": np.ascontiguousarray(b_qk.astype(np.float32)),
            "wp_l": np.ascontiguousarray(wp_l),
        })

    nc = _get_nc()
    kwargs = {}
    if _trace:
        kwargs = dict(trace=True, trace_cores=[0])
    res = run_bass_kernel_spmd(nc, in_maps, core_ids=list(range(NCORES)), **kwargs)

    # V-bias folds into the output bias because softmax rows sum to 1.
    bias_total = b_proj + b_attn[2 * C:3 * C] @ W_proj
    out = np.empty((B, T, C), dtype=np.float32)
    for b in range(B):
        acc = res.results[4 * b]["out_T"].astype(np.float32)
        for hg in range(1, 4):
            acc = acc + res.results[4 * b + hg]["out_T"].astype(np.float32)
        out[b] = acc.T + bias_total[None, :]
    if _trace:
        return out, res
    return out


# revision 11
# speedup vs baseline: 1.0998x; 1.0064x over previous
"""Causal self-attention (B=2, T=2048, C=1024, H=16) on 8 trn2 NeuronCores.

Sharding: data-parallel over B (2) x tensor-parallel over head groups (4),
so each of the 8 cores handles one batch element and 4 heads end-to-end:
QKV projection (its W_attn column slice), full-T causal attention for its
4 heads, and the partial output projection (its W_proj row slice). The
host sums the 4 per-batch partials and adds biases.

Device dataflow (all matmuls bf16; host pre-transposes and pre-casts):
  x^T uploaded directly as bf16 [C, T] (no on-device transpose)
  QKV(tg): Q^T/K^T ([d, t] bf16) and V ([t, d] bf16) per 512-token group
  S^T[k, q] = K^T.T @ Q^T per head (causal block-skipped + trimmed)
  P = exp(S/8) on ScalarE (bf16), diagonal-block mask on VectorE
  y^T = (V|1).T @ P^T accumulated in PSUM (rowsum rides along)
  y^T *= 1/rowsum (reciprocal batched via partition-reshape DMAs)
  out^T = Wp_local.T @ y^T -> DRAM bf16 [1024, 2048] per core

Software pipeline (qi-outer): QKV(tg+1) and proj(tg-1) matmuls are
interleaved as filler into attention(qi=tg)'s S->exp->PV loop so the PE
never starves while ScalarE runs exp.
"""

import numpy as np
import ml_dtypes

import concourse.bass as bass
import concourse.mybir as mybir
import concourse.tile as tile
from concourse import bacc
from concourse.bass_utils import run_bass_kernel_spmd

F32 = mybir.dt.float32
F32R = mybir.dt.float32r
BF16 = mybir.dt.bfloat16
AF = mybir.ActivationFunctionType

B, T, C, H = 2, 2048, 1024, 16
HD = C // H          # 64
NCORES = 8
CTILES = C // 128    # 8 contraction chunks
TT = T // 128        # 16 token tiles of 128
QG = T // 512        # 4 q-groups of 512
SCL = 1.0 / float(np.sqrt(HD))


def build_nc():
    nc = bacc.Bacc("TRN2", target_bir_lowering=False)

    xT_d = nc.declare_dram_parameter("xT_b", [C, T], BF16, isOutput=False)
    w_d = nc.declare_dram_parameter("w_l", [C, 768], BF16, isOutput=False)
    bqk_d = nc.declare_dram_parameter("b_qk", [4, 128], F32, isOutput=False)
    wp_d = nc.declare_dram_parameter("wp_l", [256, C], BF16, isOutput=False)
    out_d = nc.declare_dram_parameter("out_T", [C, T], BF16, isOutput=True)

    with tile.TileContext(nc) as tc:
        with tc.tile_pool(name="persist", bufs=1) as pp:
            _build_body(nc, tc, pp, xT_d, w_d, bqk_d, wp_d, out_d)
    nc.compile()
    return nc


def _build_body(nc, tc, pp, xT_d, w_d, bqk_d, wp_d, out_d):
    # ---- constants ----
    # causal mask for S^T diagonal blocks: keep where q(col) >= k(row)
    m0 = pp.tile([128, 128], BF16, tag="m0")
    nc.gpsimd.memset(m0, 1.0)
    nc.gpsimd.affine_select(out=m0, in_=m0, compare_op=mybir.AluOpType.is_ge,
                            fill=0.0, base=0, pattern=[[1, 128]], channel_multiplier=-1)

    b_sb = pp.tile([128, 4], F32, tag="b_sb")
    scr = pp.tile([1, 4], F32, tag="scr")
    nc.scalar.activation(scr, scr, AF.Exp, scale=1.0)  # preload act table early

    # ---- persistent tiles ----
    wt_all = pp.tile([128, CTILES * 768], BF16, tag="wt_all")
    wt = [wt_all[:, 768 * ci:768 * (ci + 1)] for ci in range(CTILES)]
    wp_all = pp.tile([128, 2 * C], BF16, tag="wp_all")
    wp = [wp_all[:, C * k:C * (k + 1)] for k in range(2)]
    xT_all = pp.tile([128, CTILES * T], BF16, tag="xT_all")
    xT = [xT_all[:, T * ci:T * (ci + 1)] for ci in range(CTILES)]
    qk_pair = [pp.tile([128, T], BF16, tag=f"qkp{j}", name=f"qkp{j}") for j in range(4)]
    vt = [pp.tile([128, 260], BF16, tag=f"v{t}", name=f"v{t}") for t in range(TT)]
    y_un = [pp.tile([128, T], BF16, tag=f"y{hp}", name=f"y{hp}") for hp in range(2)]

    for t in range(TT):
        g = vt[t].rearrange("p (g c) -> p g c", c=130)
        nc.vector.memset(g[:, :, 64:65], 1.0)   # even heads: ones col last
        nc.vector.memset(g[:, :, 65:66], 1.0)   # odd heads: ones col first

    def QT(h):
        lo = 64 * (h % 2)
        return qk_pair[h // 2][lo:lo + 64, :]

    def KT(h):
        lo = 64 * (h % 2)
        return qk_pair[2 + h // 2][lo:lo + 64, :]

    def VT(t, h):
        g = h // 2
        if h % 2 == 0:
            return vt[t][:, 130 * g:130 * g + 65]       # [dims64 | ones]
        return vt[t][:, 130 * g + 65:130 * g + 130]     # [ones | dims64]

    def load_xT(tg):
        nc.sync.dma_start(
            out=xT_all.rearrange("p (ci t) -> p ci t", ci=CTILES)[:, :, 512 * tg:512 * (tg + 1)],
            in_=xT_d.ap().rearrange("(ci p) t -> p ci t", ci=CTILES)[:, :, 512 * tg:512 * (tg + 1)])

    # startup DMAs: small constants, then first weight/x chunks so QKV(0)
    # starts early, then the rest
    wre = wt_all.rearrange("p (ci j) -> p ci j", ci=CTILES)
    wsrc = w_d.ap().rearrange("(ci p) j -> p ci j", ci=CTILES)
    xre = xT_all.rearrange("p (ci t) -> p ci t", ci=CTILES)
    xsrc = xT_d.ap().rearrange("(ci p) t -> p ci t", ci=CTILES)
    nc.sync.dma_start(out=wre[:, 0:2, :], in_=wsrc[:, 0:2, :])
    nc.sync.dma_start(out=xre[:, 0:2, 0:512], in_=xsrc[:, 0:2, 0:512])
    nc.sync.dma_start(out=b_sb, in_=bqk_d.ap().rearrange("j p -> p j"))
    nc.sync.dma_start(out=wre[:, 2:CTILES, :], in_=wsrc[:, 2:CTILES, :])
    nc.sync.dma_start(out=xre[:, 2:5, 0:512], in_=xsrc[:, 2:5, 0:512])
    nc.sync.dma_start(out=xre[:, 5:CTILES, 0:512], in_=xsrc[:, 5:CTILES, 0:512])
    nc.sync.dma_start(out=wp_all.rearrange("p (k j) -> p k j", k=2),
                      in_=wp_d.ap().rearrange("(k p) j -> p k j", k=2))

    with tc.tile_pool(name="qv", bufs=2, space="PSUM") as qv_pool, \
         tc.tile_pool(name="s", bufs=2, space="PSUM") as s_pool, \
         tc.tile_pool(name="y", bufs=1, space="PSUM") as y_pool, \
         tc.tile_pool(name="pb", bufs=5) as p_pool, \
         tc.tile_pool(name="stg", bufs=2) as st_pool, \
         tc.tile_pool(name="rr", bufs=2) as rr_pool, \
         tc.tile_pool(name="osb", bufs=8) as osb_pool:

        state = {}

        def qkv_steps(tg, jc_order=(0, 1, 2, 3)):
            """One step per contraction-chunk matmul; Q/K blocks then V tiles."""
            steps = []
            for jc in jc_order:
                for ci in range(CTILES):
                    def step(jc=jc, ci=ci, tg=tg):
                        if ci == 0:
                            state[("qk", jc)] = qv_pool.tile([128, 512], F32, tag="qv", name=f"qk{jc}")
                        ps = state[("qk", jc)]
                        nc.tensor.matmul(ps, wt[ci][:, 128 * jc:128 * (jc + 1)],
                                         xT[ci][:, 512 * tg:512 * (tg + 1)],
                                         start=(ci == 0), stop=(ci == CTILES - 1))
                        if ci == CTILES - 1:
                            nc.scalar.activation(qk_pair[jc][:, 512 * tg:512 * (tg + 1)],
                                                 ps, AF.Identity,
                                                 bias=b_sb[:, jc:jc + 1], scale=1.0)
                    steps.append(step)
            for tq in range(4):
                t = 4 * tg + tq
                for ci in range(CTILES):
                    def step(t=t, ci=ci):
                        if ci == 0:
                            state[("v", t)] = qv_pool.tile([128, 512], F32, tag="qv", name=f"pv{t}")
                        pv = state[("v", t)]
                        nc.tensor.matmul(pv[:, 0:256], xT[ci][:, 128 * t:128 * (t + 1)],
                                         wt[ci][:, 512:768],
                                         start=(ci == 0), stop=(ci == CTILES - 1))
                        if ci == CTILES - 1:
                            g = vt[t].rearrange("p (g c) -> p g c", c=130)
                            pg = pv[:, 0:256].rearrange("p (g c) -> p g c", c=128)
                            nc.vector.tensor_copy(g[:, :, 0:64], pg[:, :, 0:64])
                            nc.vector.tensor_copy(g[:, :, 66:130], pg[:, :, 64:128])
                    steps.append(step)
            return steps

        def proj_steps(tg, fine=False):
            steps = []
            for cg in range(2):
                def alloc(cg=cg):
                    state[("osb", cg)] = osb_pool.tile([128, 2048], BF16, tag="osb",
                                                       name=f"osb{cg}")
                for cq in range(4):
                    co = 4 * cg + cq
                    def mm(co=co, tg=tg, cg=cg, cq=cq, alloc=alloc, fine=fine):
                        if cq == 0:
                            alloc()
                        pr = qv_pool.tile([128, 512], F32, tag="qv", name=f"pr{co}")
                        state[("pr", co)] = pr
                        nc.tensor.matmul(pr, wp[0][:, 128 * co:128 * (co + 1)],
                                         y_un[0][:, 512 * tg:512 * (tg + 1)],
                                         start=True, stop=False)
                        nc.tensor.matmul(pr, wp[1][:, 128 * co:128 * (co + 1)],
                                         y_un[1][:, 512 * tg:512 * (tg + 1)],
                                         start=False, stop=True)

                    def wr(co=co, tg=tg, cg=cg, cq=cq, fine=fine):
                        pr = state[("pr", co)]
                        osb = state[("osb", cg)]
                        dst = osb[:, 512 * cq:512 * (cq + 1)]
                        nc.vector.tensor_copy(dst, pr)
                        if fine:
                            nc.sync.dma_start(
                                out=out_d[128 * co:128 * (co + 1),
                                          512 * tg:512 * (tg + 1)],
                                in_=dst)
                        elif cq == 3:
                            nc.sync.dma_start(
                                out=out_d.ap().rearrange("(j p) t -> p j t", j=CTILES)[
                                    :, 4 * cg:4 * cg + 4, 512 * tg:512 * (tg + 1)],
                                in_=osb.rearrange("p (j t) -> p j t", j=4))
                    steps.append(mm)
                    steps.append(wr)
            return steps

        def emit_pv(psA, psB, ki, pAB, hA, hB, woff, st, sp):
            nc.tensor.matmul(psA[0:65, woff:512], VT(ki, hA),
                             pAB[:, woff:512], start=st, stop=sp)
            nc.tensor.matmul(psB[0:65, woff:512], VT(ki, hB),
                             pAB[:, 512 + woff:1024], start=st, stop=sp)

        def epilogue(qi, hp, psA, psB):
            # head A: rowsum rides at psum row 64 -> recip -> bounce to row 0
            # via tiny DMA -> Pool broadcast. head B: ones-first layout puts
            # its rowsum at psum row 0 -> recip straight into the broadcast
            # source, no DMA. One fused psum*recip multiply per head.
            rr = rr_pool.tile([128, 512], F32, tag="rr")
            rr0 = rr_pool.tile([1, 1024], F32, tag="rr0")
            with nc.allow_low_precision(reason="softmax denominator reciprocal"):
                nc.vector.reciprocal(rr[64:65, :], psA[64:65, :])
                nc.vector.reciprocal(rr0[0:1, 512:1024], psB[0:1, :])
            nc.sync.dma_start(out=rr0[0:1, 0:512], in_=rr[64:65, :])
            rbA = rr_pool.tile([128, 512], F32, tag="rbA")
            rbB = rr_pool.tile([128, 512], F32, tag="rbB")
            nc.gpsimd.partition_broadcast(rbA[0:64, :], rr0[0:1, 0:512], channels=64)
            nc.gpsimd.partition_broadcast(rbB[0:65, :], rr0[0:1, 512:1024], channels=65)
            stB = st_pool.tile([128, 512], BF16, tag="st", name="stB")
            with nc.allow_low_precision(reason="softmax normalize in bf16"):
                nc.vector.tensor_mul(y_un[hp][0:64, 512 * qi:512 * (qi + 1)],
                                     psA[0:64, :], rbA[0:64, :])
                nc.vector.tensor_mul(stB[0:65, :], psB[0:65, :], rbB[0:65, :])
            nc.sync.dma_start(out=y_un[hp][64:128, 512 * qi:512 * (qi + 1)],
                              in_=stB[1:65, :])

        def attention(qi, filler):
            nkt = 4 * qi + 4
            slots = 2 * nkt
            nf = len(filler)
            prog = {"done": 0, "slot": 0}

            def pop_for_slot():
                prog["slot"] += 1
                target = (nf * prog["slot"]) // slots
                while prog["done"] < target:
                    filler[prog["done"]]()
                    prog["done"] += 1

            for hp in range(2):
                hA, hB = 2 * hp, 2 * hp + 1
                psA = y_pool.tile([128, 512], F32, tag="yA")
                psB = y_pool.tile([128, 512], F32, tag="yB")
                pend = []
                for ki in range(nkt):
                    r = ki - 4 * qi
                    soff = 0 if r < 1 else 128 * r
                    sAB = s_pool.tile([128, 1024], F32, tag="s")
                    for half, h in ((0, hA), (1, hB)):
                        nc.tensor.matmul(
                            sAB[:, 512 * half + soff:512 * half + 512],
                            KT(h)[:, 128 * ki:128 * (ki + 1)],
                            QT(h)[:, 512 * qi + soff:512 * (qi + 1)],
                            start=True, stop=True)
                    pAB = p_pool.tile([128, 1024], BF16, tag="p")
                    if r >= 1:
                        we = 128 * r
                        nc.scalar.activation(
                            pAB.rearrange("p (h q) -> p h q", h=2)[:, :, we:512],
                            sAB.rearrange("p (h q) -> p h q", h=2)[:, :, we:512],
                            AF.Exp, scale=SCL)
                    else:
                        nc.scalar.activation(pAB, sAB, AF.Exp, scale=SCL)
                    if r >= 0:
                        for half in range(2):
                            base = 512 * half + 128 * r
                            nc.vector.tensor_mul(pAB[:, base:base + 128],
                                                 pAB[:, base:base + 128], m0)
                    pop_for_slot()
                    if len(pend) >= 2:
                        emit_pv(*pend.pop(0))
                    pend.append((psA, psB, ki, pAB, hA, hB, 0 if r < 0 else 128 * r,
                                 ki == 0, ki == nkt - 1))
                for p in pend:
                    emit_pv(*p)
                epilogue(qi, hp, psA, psB)

        # ---- pipelined main sequence ----
        for step in qkv_steps(0, jc_order=(0, 2, 1, 3)):
            step()
        load_xT(1)
        filler_map = {
            0: lambda: qkv_steps(1),
            1: lambda: qkv_steps(2),
            2: lambda: qkv_steps(3) + proj_steps(0),
            3: lambda: proj_steps(1) + proj_steps(2),
        }
        for qi in range(QG):
            if qi + 2 < QG:
                load_xT(qi + 2)
            attention(qi, filler_map[qi]())
        for step in proj_steps(QG - 1, fine=True):
            step()


_NC = None


def _get_nc():
    global _NC
    if _NC is None:
        _NC = build_nc()
    return _NC


def kernel(x, W_attn, b_attn, W_proj, b_proj, _trace=False):
    x = np.asarray(x, dtype=np.float32)
    W_attn = np.asarray(W_attn, dtype=np.float32)
    b_attn = np.asarray(b_attn, dtype=np.float32)
    W_proj = np.asarray(W_proj, dtype=np.float32)
    b_proj = np.asarray(b_proj, dtype=np.float32)

    BF = ml_dtypes.bfloat16
    xTs = [np.ascontiguousarray(x[b].T.astype(BF)) for b in range(B)]
    in_maps = []
    for core in range(NCORES):
        b, hg = divmod(core, 4)
        qs = [W_attn[:, 64 * (4 * hg + h):64 * (4 * hg + h + 1)] for h in range(4)]
        ks = [W_attn[:, C + 64 * (4 * hg + h):C + 64 * (4 * hg + h + 1)] for h in range(4)]
        vs = [W_attn[:, 2 * C + 64 * (4 * hg + h):2 * C + 64 * (4 * hg + h + 1)] for h in range(4)]
        w_l = np.concatenate(qs + ks + vs, axis=1).astype(BF)
        bq = [b_attn[64 * (4 * hg + h):64 * (4 * hg + h + 1)] for h in range(4)]
        bk = [b_attn[C + 64 * (4 * hg + h):C + 64 * (4 * hg + h + 1)] for h in range(4)]
        b_qk = np.stack([np.concatenate(bq[0:2]), np.concatenate(bq[2:4]),
                         np.concatenate(bk[0:2]), np.concatenate(bk[2:4])])
        wp_l = np.concatenate(
            [W_proj[64 * (4 * hg + h):64 * (4 * hg + h + 1), :] for h in range(4)],
            axis=0).astype(BF)
        in_maps.append({
            "xT_b": xTs[b],
            "w_l": np.ascontiguousarray(w_l),
            "b_qk": np.ascontiguousarray(b_qk.astype(np.float32)),
            "wp_l": np.ascontiguousarray(wp_l),
        })

    nc = _get_nc()
    kwargs = {}
    if _trace:
        kwargs = dict(trace=True, trace_cores=[0])
    res = run_bass_kernel_spmd(nc, in_maps, core_ids=list(range(NCORES)), **kwargs)

    # V-bias folds into the output bias because softmax rows sum to 1.
    bias_total = b_proj + b_attn[2 * C:3 * C] @ W_proj
    out = np.empty((B, T, C), dtype=np.float32)
    for b in range(B):
        acc = res.results[4 * b]["out_T"].astype(np.float32)
        for hg in range(1, 4):
            acc = acc + res.results[4 * b + hg]["out_T"].astype(np.float32)
        out[b] = acc.T + bias_total[None, :]
    if _trace:
        return out, res
    return out
